# revision 40
# baseline (speedup 1.0000x reference)
"""Multi-head causal attention (B=2, S=2048, D=1024, H=16) on 8 Trainium2
NeuronCores.

Sharding: data-parallel over the 2 batches x tensor-parallel over 4 head
groups (4 heads each).  Core c handles batch c//4, heads [4*(c%4), 4*(c%4)+4).
Each core computes its Q/K/V projections from column shards of Wq/Wk/Wv,
runs causal attention for its heads, and applies its row shard of Wo,
producing a partial (D, S) output.  The host sums the 4 partials per batch
and adds the output bias.

On-core layout: activations are kept transposed (feature dim on SBUF
partitions, sequence on the free axis) so every matmul's operands are
already in the (K x M)/(K x N) form the PE array wants; the softmax
denominator comes free from an extra ones-row appended to V.

v3 structure (vs v2):
 - softmax reciprocal on the Scalar engine as exp(-ln(x)) (both funcs in
   the natural_log_exp_and_others table set) instead of the 4 us DVE
   iterative reciprocal; output directly bf16.
 - causal masking via precomputed multiplicative bf16 mask tiles applied
   with a ~0.2 us DVE tensor_mul instead of a ~1.4 us GPSIMD
   affine_select per diagonal group (the masks themselves are built once
   at startup with affine_select, which doubles as the ucode warmup).
 - last query block's normalize is fused (PSUM pv tile x broadcast recip
   -> attn_fin in one DVE op) to shorten the kernel tail.
 - head DMAs interleave weights with their x tensor (wq,bq,xq, wk,bk,xk,
   wv,bv,xv, wo) so the first projection unblocks earliest.
 - fills rebalanced so the PE has metered work across every attention
   block boundary.
"""

import sys

sys.path.insert(0, "/opt/trn_rl_repo")

import numpy as np

B, S, D, H = 2, 2048, 1024, 16
DK = D // H            # 64 head dim
NCORES = 8
NGROUPS = 4            # head groups (tensor parallel)
NH = H // NGROUPS      # 4 heads per core
DHL = NH * DK          # 256 local head dims per core
P = 128
DC = D // P            # 8 contraction chunks over D
HC = DHL // P          # 2 local head-dim chunks
SB = 512               # query block (matmul moving free size)
NSB = S // SB          # 4
SBH = 1024             # DMA/projection sequence chunk
NSBH = S // SBH        # 2
SCK = S // P           # 16 key chunks
G = 2                  # score chunks per exp group (2 PSUM banks)

_CACHE = {}


def _patch_act_tables():
    """Force Exp to resolve from the natural_log_exp_and_others table set
    (which also holds Ln) so the exp<->recip interleave doesn't thrash
    ACT_TABLE_LOADs (~1.3us each).  Only set membership used for set
    *choice* is filtered; names/order (the act_func_set_id space) are
    unchanged."""
    import functools
    import concourse.hw_specs as hw
    import concourse.bacc as bacc_mod
    import concourse.mybir as mybir

    orig = hw.get_activation_tables
    if getattr(orig, "_exp_pinned", False):
        return
    uncached = getattr(orig, "__wrapped__", orig)

    @functools.cache
    def patched(module_arch):
        t = dict(uncached(module_arch))
        Exp = mybir.ActivationFunctionType.Exp
        nl = "natural_log_exp_and_others"
        if nl in t and Exp in t[nl]:
            for name in list(t):
                if name != nl:
                    t[name] = t[name] - {Exp}
        return t

    patched._exp_pinned = True
    hw.get_activation_tables = patched
    bacc_mod.get_activation_tables = patched


def _build_nc(causal):
    import concourse.bass as bass
    import concourse.bacc as bacc
    import concourse.mybir as mybir
    import concourse.tile as tile
    from contextlib import ExitStack

    _patch_act_tables()

    f32 = mybir.dt.float32
    mmdt = mybir.dt.bfloat16
    Exp = mybir.ActivationFunctionType.Exp
    Ln = mybir.ActivationFunctionType.Ln
    is_ge = mybir.AluOpType.is_ge

    nc = bacc.Bacc(None, target_bir_lowering=False, debug=False)

    xq_d = nc.dram_tensor("xq_t", [D, S], mmdt, kind="ExternalInput")
    xk_d = nc.dram_tensor("xk_t", [D, S], mmdt, kind="ExternalInput")
    xv_d = nc.dram_tensor("xv_t", [D, S], mmdt, kind="ExternalInput")
    # weights pre-arranged on host to the exact SBUF layouts
    wq_d = nc.dram_tensor("wq_a", [P, DC * DHL], mmdt, kind="ExternalInput")
    wk_d = nc.dram_tensor("wk_a", [P, DC * DHL], mmdt, kind="ExternalInput")
    wv_d = nc.dram_tensor("wv_a", [P, DC * DHL], mmdt, kind="ExternalInput")
    wo_d = nc.dram_tensor("wo_a", [P, HC * D], mmdt, kind="ExternalInput")
    bq_d = nc.dram_tensor("bq_a", [P, HC], f32, kind="ExternalInput")
    bk_d = nc.dram_tensor("bk_a", [P, HC], f32, kind="ExternalInput")
    bv_d = nc.dram_tensor("bv_a", [1, DHL], f32, kind="ExternalInput")
    out_d = nc.dram_tensor("out_t", [D, S], mmdt, kind="ExternalOutput")

    inv_sqrt_dk = 1.0 / float(np.sqrt(DK))

    with tile.TileContext(nc) as tc, ExitStack() as ctx:
        consts = ctx.enter_context(tc.tile_pool(name="consts", bufs=1))
        xpool = ctx.enter_context(tc.tile_pool(name="xpool", bufs=2))
        ex_pool = ctx.enter_context(tc.tile_pool(name="ex_pool", bufs=8))
        small = ctx.enter_context(tc.tile_pool(name="small", bufs=2))
        opool = ctx.enter_context(tc.tile_pool(name="opool", bufs=4))
        proj_ps = ctx.enter_context(
            tc.tile_pool(name="proj_ps", bufs=2, space="PSUM"))
        sc_ps = ctx.enter_context(
            tc.tile_pool(name="sc_ps", bufs=2, space="PSUM"))
        pv_ps = ctx.enter_context(
            tc.tile_pool(name="pv_ps", bufs=2, space="PSUM"))

        # --- resident tensors ---
        wq_sb = consts.tile([P, DC, DHL], mmdt)
        wk_sb = consts.tile([P, DC, DHL], mmdt)
        wv_sb = consts.tile([P, DC, DHL], mmdt)
        wo_sb = consts.tile([P, HC, D], mmdt)
        bq_sb = consts.tile([P, HC], f32)
        bk_sb = consts.tile([P, HC], f32)
        bv_row = consts.tile([1, DHL], f32)
        bv_bc = consts.tile([P, DHL], f32)
        q_sb = consts.tile([P, HC, S], mmdt)
        k_sb = consts.tile([P, HC, S], mmdt)
        v_aug = consts.tile([P, SCK, NH, DK + 1], mmdt)
        attn_sb = consts.tile([P, HC, S], mmdt)   # unnormalized PV
        attn_fin = consts.tile([P, HC, S], mmdt)  # normalized
        ones_sb = consts.tile([P, SB], f32)
        ones_row = consts.tile([1, P], mmdt)  # rank-1 broadcast stationary
        # multiplicative causal masks for the two diagonal group offsets:
        # gm[d0][k, j, q] = 1 iff q >= k + 128*(d0+j)
        gm = {d0: consts.tile([P, G, SB], mmdt, name=f"gm{d0}")
              for d0 in (0, 2)}

        xts_by_sbh = {}

        def load_x_half(sbh, name, x_d, hi):
            """DMA one dc-half of an x tensor; allocates the tile on the
            first half so weight DMAs can interleave between halves."""
            hs = slice(sbh * SBH, (sbh + 1) * SBH)
            if hi == 0:
                xt = xpool.tile([P, DC, SBH], mmdt, name=f"x{name}{sbh}",
                                tag=f"x{name}")
                xts_by_sbh.setdefault(sbh, {})[name] = xt
            else:
                xt = xts_by_sbh[sbh][name]
            src = x_d[:].rearrange("(c p) s -> p c s", p=P)[:, :, hs]
            half = DC // 2
            nc.sync.dma_start(xt[:, hi * half:(hi + 1) * half, :],
                              src[:, hi * half:(hi + 1) * half, :])

        def load_x_tensor(sbh, name, x_d):
            load_x_half(sbh, name, x_d, 0)
            load_x_half(sbh, name, x_d, 1)

        def emit_head_dmas():
            # ordered so the first Q projection matmuls unblock earliest
            # and each later weight lands BEFORE its x data (the weight
            # otherwise gates that projection long after its x arrived)
            nc.sync.dma_start(
                wq_sb[:], wq_d[:].rearrange("p (c h) -> p c h", c=DC))
            nc.sync.dma_start(bq_sb[:], bq_d[:])
            # first xq half in dc-quarters: the very first projection
            # matmuls unblock ~1.5us earlier
            xt_q = xpool.tile([P, DC, SBH], mmdt, name="xq0", tag="xq")
            xts_by_sbh.setdefault(0, {})["q"] = xt_q
            src_q = xq_d[:].rearrange("(c p) s -> p c s", p=P)[:, :, 0:SBH]
            nc.sync.dma_start(xt_q[:, 0:2, :], src_q[:, 0:2, :])
            nc.sync.dma_start(xt_q[:, 2:4, :], src_q[:, 2:4, :])
            load_x_half(0, "q", xq_d, 1)
            nc.sync.dma_start(
                wk_sb[:], wk_d[:].rearrange("p (c h) -> p c h", c=DC))
            nc.sync.dma_start(bk_sb[:], bk_d[:])
            load_x_half(0, "k", xk_d, 0)
            nc.sync.dma_start(
                wv_sb[:], wv_d[:].rearrange("p (c h) -> p c h", c=DC))
            nc.sync.dma_start(bv_row[:], bv_d[:])
            load_x_half(0, "k", xk_d, 1)
            nc.sync.dma_start(
                wo_sb[:], wo_d[:].rearrange("p (c o) -> p c o", c=HC))
            load_x_half(0, "v", xv_d, 0)
            load_x_half(0, "v", xv_d, 1)
            nc.gpsimd.partition_broadcast(bv_bc[:], bv_row[:])
            ones_f = consts.tile([P, SCK * NH], f32)
            nc.gpsimd.memset(ones_f[:], 1.0)
            nc.vector.tensor_copy(
                v_aug[:, :, :, DK],
                ones_f[:].rearrange("p (a b) -> p a b", a=SCK))
            nc.gpsimd.memset(ones_sb[:], 1.0)
            nc.gpsimd.memset(ones_row[:], 1.0)
            # build the causal mask tiles with affine_select on a ones
            # tile; this also warms the GPSIMD ucode libraries during the
            # DMA-bound head (combined IRAM library load would otherwise
            # stall every engine ~7us at first use)
            ones_bf = consts.tile([P, G, SB], mmdt)
            nc.gpsimd.memset(ones_bf[:], 1.0)
            for d0 in (0, 2):
                nc.gpsimd.affine_select(
                    gm[d0][:], ones_bf[:],
                    pattern=[[-P, G], [1, SB]],
                    compare_op=is_ge, fill=0.0,
                    base=-P * d0, channel_multiplier=-1)
            # warm the ACT function-table load (~2.7us) during the
            # DMA-bound head instead of stalling the first softmax exp
            act_warm = small.tile([1, 8], f32, name="act_warm", tag="warm")
            nc.scalar.activation(act_warm[:], ones_sb[0:1, 0:8], Exp)
            nc.scalar.activation(act_warm[:], act_warm[:], Ln)

        bv_bc_r = bv_bc[:].rearrange("p (h e) -> p h e", h=NH)

        def recip_act(dst, src, key):
            """dst = 1/src via exp(-ln(src)) on the Scalar engine."""
            ln_t = small.tile([P, SB], f32, name=f"ln{key}", tag="lnt")
            nc.scalar.activation(ln_t[:], src[:], Ln)
            nc.scalar.activation(dst[:], ln_t[:], Exp, bias=0.0, scale=-1.0)

        def emit_attention(qb, fills):
            """Emit attention for query block qb, interleaving `fills`
            (projection / output-projection PSUM-group emitters) between
            attention groups so the PE always has metered fallback work
            while the Scalar engine paces the softmax."""
            def fill_one():
                if fills:
                    fills.pop(0)()
            n_chunks = (qb + 1) * (SB // P) if causal else SCK
            n_groups = n_chunks // G
            qs = slice(qb * SB, (qb + 1) * SB)
            last = qb == NSB - 1
            den4 = None
            if not last:
                # denominators for the 4 heads live at partitions
                # 0/32/64/96 (partition bases must be 32-aligned)
                den4 = small.tile([P, SB], f32, name=f"den{qb}", tag="den")
            for p in range(2):  # head pair == hc index
                pv_t = [
                    pv_ps.tile([DK + 1, SB], f32, name=f"pv{qb}_{p}_{h2}",
                               tag="pv")
                    for h2 in range(2)
                ]
                for g in range(n_groups):
                    # diagonal chunks only need queries >= (tj - qb*4)*128:
                    # slice the moving operand (ragged causal trimming)
                    def chunk_qoff(tj):
                        return max(0, tj - qb * (SB // P)) * P if causal \
                            else 0
                    goff = chunk_qoff(g * G)
                    sc_ts = [
                        sc_ps.tile([P, G, SB], f32, name="sc", tag="sc")
                        for _ in range(2)
                    ]
                    # both heads' score matmuls adjacent: disjoint 64-row
                    # groups of the PE array -> run concurrently
                    for j2 in range(G):
                        tj = g * G + j2
                        qo = chunk_qoff(tj)
                        for h2 in range(2):
                            po = h2 * DK
                            nc.tensor.matmul(
                                sc_ts[h2][:, j2, qo:],
                                k_sb[po:po + DK, p, tj * P:(tj + 1) * P],
                                q_sb[po:po + DK, p,
                                     qb * SB + qo:(qb + 1) * SB],
                                start=True, stop=True)
                    ex_ts = []
                    for h2 in range(2):
                        ex = ex_pool.tile([P, G, SB], mmdt, name="ex",
                                          tag="ex")
                        nc.scalar.activation(
                            ex[:, :, goff:], sc_ts[h2][:, :, goff:], Exp,
                            bias=0.0, scale=inv_sqrt_dk)
                        if causal and g * G + G > qb * (SB // P):
                            d0 = g * G - qb * (SB // P)
                            nc.vector.tensor_mul(
                                ex[:, :, goff:], ex[:, :, goff:],
                                gm[d0][:, :, goff:])
                        ex_ts.append(ex)
                    for h2 in range(2):
                        hl = 2 * p + h2
                        for j2 in range(G):
                            tj = g * G + j2
                            qo = chunk_qoff(tj)
                            nc.tensor.matmul(
                                pv_t[h2][:, qo:], v_aug[:, tj, hl, :],
                                ex_ts[h2][:, j2, qo:],
                                start=(g == 0 and j2 == 0),
                                stop=(g == n_groups - 1 and j2 == G - 1))
                    fill_one()
                if last and p == 0:
                    # last block, pair 0: two-pass through attn_sb so the
                    # pv PSUM slots free immediately and pair 1's PV is
                    # never blocked; broadcast via GPSIMD (runs while
                    # pair 1's attention occupies PE/ACT).
                    den2 = small.tile([P, SB], f32, name=f"den{qb}_{p}",
                                      tag="den")
                    nc.vector.tensor_copy(
                        attn_sb[0:DK, p, qs], pv_t[0][0:DK, :])
                    nc.vector.tensor_mul(
                        attn_sb[DK:P, p, qs], pv_t[1][0:DK, :],
                        ones_sb[0:DK, :])
                    for h2 in range(2):
                        nc.vector.tensor_mul(
                            den2[32 * h2:32 * h2 + 1, :],
                            pv_t[h2][DK:DK + 1, :], ones_sb[DK:DK + 1, :])
                    recip2 = small.tile([P, SB], mmdt, name=f"recip{qb}_{p}",
                                        tag="recip4")
                    recip_act(recip2, den2, f"{qb}_{p}")
                    for h2 in range(2):
                        hl = 2 * p + h2
                        po = h2 * DK
                        rN = small.tile([1, SB], mmdt, name=f"rN{qb}_{hl}",
                                        tag="recipN")
                        nc.vector.tensor_copy(
                            rN[0:1, :], recip2[32 * h2:32 * h2 + 1, :])
                        rbc = small.tile([P, SB], mmdt, name=f"rbc{qb}_{hl}",
                                         tag="rbc")
                        nc.gpsimd.partition_broadcast(rbc[:], rN[0:1, :])
                        nc.vector.tensor_mul(
                            attn_fin[po:po + DK, p, qs],
                            attn_sb[po:po + DK, p, qs], rbc[po:po + DK, :])
                    fill_one()
                elif last:
                    # last block, last pair: finalize deferred until the
                    # remaining fills drained (below), so the PE FIFO
                    # isn't blocked behind the recip chain
                    last_pv = pv_t
                else:
                    # cross-partition-base writes must be tensor_tensor
                    # ops: the BIR verifier requires TensorCopy in/out
                    # partitions to match, but TT outputs may sit on other
                    # partitions.
                    nc.vector.tensor_copy(
                        attn_sb[0:DK, p, qs], pv_t[0][0:DK, :])
                    nc.vector.tensor_mul(
                        attn_sb[DK:P, p, qs], pv_t[1][0:DK, :],
                        ones_sb[0:DK, :])
                    for h2 in range(2):
                        hl = 2 * p + h2
                        nc.vector.tensor_mul(
                            den4[32 * hl:32 * hl + 1, :],
                            pv_t[h2][DK:DK + 1, :], ones_sb[DK:DK + 1, :])
                    fill_one()
            if last:
                # drain leftover fills FIRST: they execute on the PE
                # while the ACT computes the reciprocals, and the rank-1
                # broadcast matmuls slot in right after
                while fills:
                    fill_one()
                # the kernel tail.  ACT reciprocal straight from the PSUM
                # denominator rows (lands at partition 0, no staging
                # copy); the pv rows are staged to attn_sb on the DVE (in
                # parallel with the ACT chain) so the final normalize mul
                # has only the rank-1-broadcast PSUM operand; warm-keeper
                # matmuls stop HAM from re-throttling the PE during the
                # chain so op(3) runs at full clock.
                p = 1
                nc.vector.tensor_copy(
                    attn_sb[0:DK, p, qs], last_pv[0][0:DK, :])
                nc.vector.tensor_mul(
                    attn_sb[DK:P, p, qs], last_pv[1][0:DK, :],
                    ones_sb[0:DK, :])
                lnNs, recNs = [], []
                for h2 in range(2):
                    lnN = small.tile([1, SB], f32, name=f"lnN{h2}",
                                     tag="lnN")
                    nc.scalar.activation(
                        lnN[0:1, :], last_pv[h2][DK:DK + 1, :], Ln)
                    lnNs.append(lnN)
                def warm_mm(w):
                    warm_t = sc_ps.tile([P, SB], f32, name=f"warm{w}",
                                        tag="sc")
                    nc.tensor.matmul(
                        warm_t[:], ones_row[0:1, :], gm[0][0:1, 0, :],
                        start=True, stop=True)
                for w in range(6):
                    warm_mm(w)
                for h2 in range(2):
                    recN = small.tile([1, SB], mmdt, name=f"recN{h2}",
                                      tag="recipN")
                    nc.scalar.activation(
                        recN[0:1, :], lnNs[h2][0:1, :], Exp,
                        bias=0.0, scale=-1.0)
                    recNs.append(recN)
                for h2 in range(2):
                    po = h2 * DK
                    rbc_t = sc_ps.tile([P, SB], f32, name=f"rbc3_{h2}",
                                       tag="sc")
                    nc.tensor.matmul(
                        rbc_t[:], ones_row[0:1, :], recNs[h2][0:1, :],
                        start=True, stop=True)
                    nc.vector.tensor_mul(
                        attn_fin[po:po + DK, p, qs],
                        attn_sb[po:po + DK, p, qs], rbc_t[po:po + DK, :])
                    if h2 == 0:
                        warm_mm(6)
                        warm_mm(7)
            else:
                recip4 = small.tile([P, SB], mmdt, name=f"recip{qb}",
                                    tag="recip4")
                recip_act(recip4, den4, f"{qb}")
                for hl in range(NH):
                    normalize_head(qb, qs, hl, recip4, 32 * hl)
                fill_one()
                while fills:
                    fill_one()

        def normalize_head(qb, qs, hl, recip_t, row):
            p_, h2 = hl // 2, hl % 2
            po = h2 * DK
            rN = small.tile([1, SB], mmdt, name=f"rN{qb}_{hl}",
                            tag="recipN")
            nc.vector.tensor_copy(rN[0:1, :], recip_t[row:row + 1, :])
            rbc = small.tile([P, SB], mmdt, name=f"rbc{qb}_{hl}",
                             tag="rbc")
            nc.gpsimd.partition_broadcast(rbc[:], rN[0:1, :])
            nc.vector.tensor_mul(
                attn_fin[po:po + DK, p_, qs],
                attn_sb[po:po + DK, p_, qs], rbc[po:po + DK, :])

        def make_proj_fills(sbi, xts):
            """PSUM-group closures for block sbi's projections: returns
            (qk, v) with qk = [q_hc0, q_hc1, k_hc0, k_hc1], v = [v0..v3]."""
            nn = sbi % (SBH // SB)
            ss = slice(sbi * SB, (sbi + 1) * SB)
            qk = []
            for name, w_sb, b_sb, t_sb in (
                    ("q", wq_sb, bq_sb, q_sb), ("k", wk_sb, bk_sb, k_sb)):
                for hc in range(HC):
                    def f(name=name, w_sb=w_sb, b_sb=b_sb, t_sb=t_sb, hc=hc):
                        xt = xts[name]
                        ps = proj_ps.tile([P, SB], f32, name="proj",
                                          tag="proj")
                        for dc in range(DC):
                            nc.tensor.matmul(
                                ps[:], w_sb[:, dc, hc * P:(hc + 1) * P],
                                xt[:, dc, nn * SB:(nn + 1) * SB],
                                start=(dc == 0), stop=(dc == DC - 1))
                        nc.vector.tensor_scalar_add(
                            t_sb[:, hc, ss], ps[:], b_sb[:, hc:hc + 1])
                    qk.append(f)
            v = []
            for scl4 in range(SB // P):
                def fv(scl4=scl4):
                    xt = xts["v"]
                    scl = nn * (SB // P) + scl4
                    sc_i = sbi * (SB // P) + scl4
                    ps = proj_ps.tile([P, DHL], f32, name="proj", tag="proj")
                    for dc in range(DC):
                        nc.tensor.matmul(
                            ps[:], xt[:, dc, scl * P:(scl + 1) * P],
                            wv_sb[:, dc, :],
                            start=(dc == 0), stop=(dc == DC - 1))
                    nc.vector.tensor_add(
                        v_aug[:, sc_i, :, 0:DK],
                        ps[:].rearrange("p (h e) -> p h e", h=NH), bv_bc_r)
                v.append(fv)
            return qk, v

        def make_oproj_fills(qb, evac_on_act=False):
            """One closure per output-projection PSUM-group of block qb.
            evac_on_act moves the PSUM->SBUF copy to the Scalar engine
            (only sensible in the kernel tail, where ACT is idle but the
            DVE is busy with the last softmax normalize)."""
            qs = slice(qb * SB, (qb + 1) * SB)
            fills = []
            for oc in range(DC):
                def f(oc=oc):
                    if evac_on_act and oc % 2 == 1:
                        # kernel tail: the score PSUM slots are idle, so
                        # alternate banks with proj_ps -> 4 groups in
                        # flight and the MM pace decouples from the
                        # evacuation pace
                        ps = sc_ps.tile([P, SB], f32, name="oproj_sc",
                                        tag="sc")
                    else:
                        ps = proj_ps.tile([P, SB], f32, name="proj",
                                          tag="proj")
                    for hc2 in range(HC):
                        nc.tensor.matmul(
                            ps[:], wo_sb[:, hc2, oc * P:(oc + 1) * P],
                            attn_fin[:, hc2, qs],
                            start=(hc2 == 0), stop=(hc2 == HC - 1))
                    o_tile = opool.tile([P, SB], mmdt, name="ot", tag="ot")
                    if evac_on_act and oc % 2 == 0:
                        nc.scalar.copy(o_tile[:], ps[:])
                    else:
                        nc.vector.tensor_copy(o_tile[:], ps[:])
                    nc.sync.dma_start(
                        out_d[:].rearrange("(c p) s -> p c s", p=P)
                        [:, oc, qs], o_tile[:])
                fills.append(f)
            return fills

        def make_qk_fills_split(sbi, xts):
            """q/k projection fills split into dc-half pairs; returns
            [q0a,q0b,q1a,q1b,k0a,k0b,k1a,k1b]."""
            nn = sbi % (SBH // SB)
            ss = slice(sbi * SB, (sbi + 1) * SB)
            out = []
            for name, w_sb, b_sb, t_sb in (
                    ("q", wq_sb, bq_sb, q_sb), ("k", wk_sb, bk_sb, k_sb)):
                for hc in range(HC):
                    cell = {}
                    def fa(name=name, w_sb=w_sb, hc=hc, cell=cell):
                        xt = xts[name]
                        ps = proj_ps.tile([P, SB], f32, name="proj",
                                          tag="proj")
                        cell["ps"] = ps
                        for dc in range(DC // 2):
                            nc.tensor.matmul(
                                ps[:], w_sb[:, dc, hc * P:(hc + 1) * P],
                                xt[:, dc, nn * SB:(nn + 1) * SB],
                                start=(dc == 0), stop=False)
                    def fb(name=name, w_sb=w_sb, b_sb=b_sb, t_sb=t_sb,
                           hc=hc, cell=cell):
                        xt = xts[name]
                        ps = cell["ps"]
                        for dc in range(DC // 2, DC):
                            nc.tensor.matmul(
                                ps[:], w_sb[:, dc, hc * P:(hc + 1) * P],
                                xt[:, dc, nn * SB:(nn + 1) * SB],
                                start=False, stop=(dc == DC - 1))
                        nc.vector.tensor_scalar_add(
                            t_sb[:, hc, ss], ps[:], b_sb[:, hc:hc + 1])
                    out += [fa, fb]
            return out

        def make_v_fills_split(sbi, xts):
            """vs fills split into dc-half pairs (one shared PSUM group)
            for finer PE metering in the exp-paced early blocks."""
            nn = sbi % (SBH // SB)
            v = []
            for scl4 in range(SB // P):
                cell = {}
                def fa(scl4=scl4, cell=cell):
                    xt = xts["v"]
                    scl = nn * (SB // P) + scl4
                    ps = proj_ps.tile([P, DHL], f32, name="proj",
                                      tag="proj")
                    cell["ps"] = ps
                    for dc in range(DC // 2):
                        nc.tensor.matmul(
                            ps[:], xt[:, dc, scl * P:(scl + 1) * P],
                            wv_sb[:, dc, :],
                            start=(dc == 0), stop=False)
                def fb(scl4=scl4, cell=cell):
                    xt = xts["v"]
                    scl = nn * (SB // P) + scl4
                    sc_i = sbi * (SB // P) + scl4
                    ps = cell["ps"]
                    for dc in range(DC // 2, DC):
                        nc.tensor.matmul(
                            ps[:], xt[:, dc, scl * P:(scl + 1) * P],
                            wv_sb[:, dc, :],
                            start=False, stop=(dc == DC - 1))
                    nc.vector.tensor_add(
                        v_aug[:, sc_i, :, 0:DK],
                        ps[:].rearrange("p (h e) -> p h e", h=NH), bv_bc_r)
                v += [fa, fb]
            return v

        def make_oproj_fills_split(qb):
            """Like make_oproj_fills but each oc group is TWO fills (one
            matmul each) for finer PE metering late in the pipeline; the
            PSUM tile is shared through a cell."""
            qs = slice(qb * SB, (qb + 1) * SB)
            fills = []
            for oc in range(DC):
                cell = {}
                def fa(oc=oc, cell=cell):
                    ps = proj_ps.tile([P, SB], f32, name="proj", tag="proj")
                    cell["ps"] = ps
                    nc.tensor.matmul(
                        ps[:], wo_sb[:, 0, oc * P:(oc + 1) * P],
                        attn_fin[:, 0, qs], start=True, stop=False)
                def fb(oc=oc, cell=cell):
                    ps = cell["ps"]
                    nc.tensor.matmul(
                        ps[:], wo_sb[:, 1, oc * P:(oc + 1) * P],
                        attn_fin[:, 1, qs], start=False, stop=True)
                    o_tile = opool.tile([P, SB], mmdt, name="ot", tag="ot")
                    nc.vector.tensor_copy(o_tile[:], ps[:])
                    nc.sync.dma_start(
                        out_d[:].rearrange("(c p) s -> p c s", p=P)
                        [:, oc, qs], o_tile[:])
                fills += [fa, fb]
            return fills

        # --- pipeline: proj(0) first, then per query block qb run
        # attention(qb) with proj(qb+1) + O-proj(qb-1) interleaved as
        # metered PE fill work; O-proj for the last blocks drains at the
        # end of the following attention block / the kernel tail ---
        emit_head_dmas()
        qks, vs = {}, {}
        qks[0], vs[0] = make_proj_fills(0, xts_by_sbh[0])
        qks[1], vs[1] = make_proj_fills(1, xts_by_sbh[0])
        # proj(0)+q(1) up front in DMA-arrival order: both blocks' q
        # fills need only xq, so the PE has stall-free work while wk/xk
        # and wv/xv are still streaming; then k block0, then v block0
        for f in [qks[0][0], qks[0][1], qks[1][0], qks[1][1],
                  qks[0][2], qks[0][3]] + vs[0]:
            f()
        op = {qb: make_oproj_fills(qb, evac_on_act=(qb == NSB - 1))
              for qb in (0, 1, NSB - 1)}
        op[2] = make_oproj_fills_split(2)
        # qks[b] layout: [q_hc0, q_hc1, k_hc0, k_hc1]
        # a(0): 7 pops, 10 fills (split v fills meter the exp-paced
        # groups finer; surplus drains over the qb0 recip chain)
        vs1s = make_v_fills_split(1, xts_by_sbh[0])
        emit_attention(0, [qks[1][2], qks[1][3]] + vs1s)
        load_x_tensor(1, "q", xq_d)
        load_x_tensor(1, "k", xk_d)
        load_x_tensor(1, "v", xv_d)
        qks[2], vs[2] = make_proj_fills(2, xts_by_sbh[1])
        qks[3], vs[3] = make_proj_fills(3, xts_by_sbh[1])
        # a(1): 11 pops, 16 fills, all split (surplus ~= qb1's
        # recip-chain window; the halves also unblock earlier on the
        # still-streaming SBH1 DMAs)
        qk2s = make_qk_fills_split(2, xts_by_sbh[1])
        vs2s = make_v_fills_split(2, xts_by_sbh[1])
        emit_attention(1, [qk2s[0], qk2s[1], qk2s[4], qk2s[5],
                           qk2s[2], qk2s[3], qk2s[6], qk2s[7]] + vs2s)
        # a(2): 15 pops, 21 fills
        vs3s = make_v_fills_split(3, xts_by_sbh[1])
        emit_attention(2, [op[0][0], op[0][1],
                           qks[3][0], qks[3][2], qks[3][1], qks[3][3]]
                       + vs3s + op[0][2:8] + op[1][0:1])
        # a(3): 18 pops, 23 fills; op(2) placed late so its attn_fin(2)
        # dependency is ready by the time the PE FIFO reaches it, and the
        # surplus drains during the last pair's finalize chain (keeps the
        # PE warm so op(3) runs at full clock)
        emit_attention(3, op[1][1:8] + op[2])
        for f in op[NSB - 1]:
            f()

    nc.compile()
    return nc


def _get_nc(causal):
    key = ("causal" if causal else "dense")
    if key not in _CACHE:
        _CACHE[key] = _build_nc(causal)
    return _CACHE[key]


def _prep_core_inputs(Q, K, V, Wq, bq, Wk, bk, Wv, bv, Wo):
    """Build the 8 per-core input maps (all arrays C-contiguous)."""
    cc = np.ascontiguousarray
    in_maps = []
    for c in range(NCORES):
        b = c // NGROUPS
        g = c % NGROUPS
        hs, he = g * DHL, (g + 1) * DHL
        # weights pre-arranged to SBUF layout [128, DC, DHL] with d = dc*128+p
        wq_a = cc(Wq[hs:he, :].T.reshape(DC, P, DHL).transpose(1, 0, 2)
                  .reshape(P, DC * DHL))
        wk_a = cc(Wk[hs:he, :].T.reshape(DC, P, DHL).transpose(1, 0, 2)
                  .reshape(P, DC * DHL))
        wv_a = cc(Wv[hs:he, :].T.reshape(DC, P, DHL).transpose(1, 0, 2)
                  .reshape(P, DC * DHL))
        # Wo shard: lhsT layout [hd, dout] split to [128, HC, D], hd = hc*128+p
        wo_a = cc(Wo[:, hs:he].T.reshape(HC, P, D).transpose(1, 0, 2)
                  .reshape(P, HC * D))
        import ml_dtypes
        bf16 = ml_dtypes.bfloat16
        in_maps.append({
            "xq_t": cc(Q[b].T).astype(bf16), "xk_t": cc(K[b].T).astype(bf16),
            "xv_t": cc(V[b].T).astype(bf16),
            "wq_a": wq_a.astype(bf16), "wk_a": wk_a.astype(bf16),
            "wv_a": wv_a.astype(bf16), "wo_a": wo_a.astype(bf16),
            "bq_a": cc(bq[hs:he].reshape(HC, P).T),
            "bk_a": cc(bk[hs:he].reshape(HC, P).T),
            "bv_a": cc(bv[hs:he].reshape(1, DHL)),
        })
    return in_maps


def _classify_mask(mask):
    m = np.asarray(mask)
    if m.dtype != np.bool_:
        m = m.astype(bool)
    causal = np.tril(np.ones((S, S), dtype=bool))
    if all(np.array_equal(m[b, 0], causal) for b in range(m.shape[0])):
        return "causal"
    if m.all():
        return "dense"
    return "generic"


def _numpy_reference(Q, K, V, mask, Wq, bq, Wk, bk, Wv, bv, Wo, bo):
    """Plain numpy fallback for arbitrary masks."""
    out = np.empty((B, S, D), dtype=np.float32)
    for b in range(B):
        q = (Q[b] @ Wq.T + bq).reshape(S, H, DK).transpose(1, 0, 2)
        k = (K[b] @ Wk.T + bk).reshape(S, H, DK).transpose(1, 0, 2)
        v = (V[b] @ Wv.T + bv).reshape(S, H, DK).transpose(1, 0, 2)
        m = np.asarray(mask[b, 0], dtype=bool)
        acc = np.empty((H, S, DK), dtype=np.float32)
        for h in range(H):
            s = (q[h] @ k[h].T) / np.float32(np.sqrt(DK))
            s = np.where(m, s, np.float32(-1e9))
            s = s - s.max(axis=-1, keepdims=True)
            e = np.exp(s)
            p = e / e.sum(axis=-1, keepdims=True)
            acc[h] = p @ v[h]
        out[b] = acc.transpose(1, 0, 2).reshape(S, D) @ Wo.T + bo
    return out


def kernel(Q, K, V, mask, Wq, bq, Wk, bk, Wv, bv, Wo, bo,
           _profile=False, _trace_dir=None):
    from concourse.bass_utils import run_bass_kernel_spmd

    flavor = _classify_mask(mask)
    if flavor == "generic":
        return _numpy_reference(Q, K, V, mask, Wq, bq, Wk, bk, Wv, bv, Wo, bo)

    nc = _get_nc(flavor == "causal")
    in_maps = _prep_core_inputs(
        np.asarray(Q, np.float32), np.asarray(K, np.float32),
        np.asarray(V, np.float32), np.asarray(Wq, np.float32),
        np.asarray(bq, np.float32), np.asarray(Wk, np.float32),
        np.asarray(bk, np.float32), np.asarray(Wv, np.float32),
        np.asarray(bv, np.float32), np.asarray(Wo, np.float32))

    kwargs = {}
    if _profile:
        import concourse.bass_utils as _bu
        _bu.upload_artifacts = lambda d: d  # no cloud copy in this container
        kwargs = dict(trace=True, trace_cores=[0])
        if _trace_dir is not None:
            kwargs["tmpdir"] = _trace_dir
    res = run_bass_kernel_spmd(nc, in_maps, core_ids=list(range(NCORES)),
                               **kwargs)

    out = np.empty((B, S, D), dtype=np.float32)
    bo32 = np.asarray(bo, np.float32)
    for b in range(B):
        acc = res.results[b * NGROUPS]["out_t"].astype(np.float32)
        for g in range(1, NGROUPS):
            acc = acc + res.results[b * NGROUPS + g]["out_t"]
        out[b] = acc.T + bo32
    if _profile:
        kernel._last_exec_time_ns = res.exec_time_ns
        kernel._last_results = res
    return out
